# revision 1
# baseline (speedup 1.0000x reference)
"""GCN link-prediction kernel for 8 Trainium2 NeuronCores.

Strategy:
  - Nodes (dst) sharded across 8 cores (12500 each); each core processes the
    edges whose dst lands in its shard (plus its self-loops).
  - GCN sym-norm factorizes per node: out[d] = dinv[d] * sum_{s in N(d)+d}
    dinv[s]*h[s].  So the per-edge norm disappears: pre-scale rows by dinv
    when writing the dense-matmul result, post-scale the aggregation by dinv.
  - Aggregation: edges sorted by dst block (128 nodes); per 128-edge chunk:
    indirect-DMA gather of h~[src] rows -> one-hot selection matrix built
    on-device from dst_local via iota/is_equal -> PE matmul accumulated in
    PSUM per dst block.
  - One compiled program serves both GCN layers (W2 zero-padded to 128 cols;
    relu vs identity via per-partition threshold input: max(v, thr)).
  - Decode runs as a second program: gather z[i], z[j] per 128-pair chunk,
    multiply + row-reduce on DVE.
Host does index-only prep (degree, sorting, padding) and inter-program
concatenation of shards.
"""
import numpy as np

import concourse.bass as bass
import concourse.bacc as bacc
import concourse.mybir as mybir
import concourse.tile as tile
from concourse.bass_utils import run_bass_kernel_spmd
from concourse.masks import make_identity

f32 = mybir.dt.float32
i32 = mybir.dt.int32

N = 100000
E = 1600000
EL = 1048576
IN = 128
HID = 128
OUT = 64
NCORES = 8
NPC = N // NCORES          # 12500 nodes per core
NBLK = (NPC + 127) // 128  # 98 dst blocks per core
P = 128

_prog_cache = {}


def _prep(edge_index):
    src = np.asarray(edge_index[0], dtype=np.int64)
    dst = np.asarray(edge_index[1], dtype=np.int64)
    deg = np.bincount(dst, minlength=N).astype(np.float64) + 1.0
    dinv = (1.0 / np.sqrt(deg)).astype(np.float32)

    # per (core, block) edge lists, self-loops included
    core_of = dst // NPC
    per_core = []
    counts = np.zeros((NCORES, NBLK), dtype=np.int64)
    for c in range(NCORES):
        m = core_of == c
        s_c = src[m]
        d_c = dst[m] - c * NPC
        loop = np.arange(NPC, dtype=np.int64)
        s_c = np.concatenate([s_c, loop + c * NPC])
        d_c = np.concatenate([d_c, loop])
        blk = d_c // 128
        order = np.argsort(blk, kind="stable")
        s_c, d_c, blk = s_c[order], d_c[order], blk[order]
        per_core.append((s_c, d_c, blk))
        counts[c] = np.bincount(blk, minlength=NBLK)

    kb = ((counts.max(axis=0) + 127) // 128).astype(np.int64)  # chunks per blk
    ktot = int(kb.sum())
    col_off = np.concatenate([[0], np.cumsum(kb)[:-1]])

    srcs = np.zeros((NCORES, 128, ktot), dtype=np.int32)
    dstl = np.full((NCORES, 128, ktot), 999.0, dtype=np.float32)
    for c in range(NCORES):
        s_c, d_c, blk = per_core[c]
        pos = 0
        for b in range(NBLK):
            m = int(counts[c, b])
            cols = int(kb[b])
            spad = np.zeros(cols * 128, dtype=np.int32)
            dpad = np.full(cols * 128, 999.0, dtype=np.float32)
            spad[:m] = s_c[pos:pos + m]
            dpad[:m] = (d_c[pos:pos + m] % 128).astype(np.float32)
            srcs[c, :, col_off[b]:col_off[b] + cols] = spad.reshape(cols, 128).T
            dstl[c, :, col_off[b]:col_off[b] + cols] = dpad.reshape(cols, 128).T
            pos += m

    # dinv arranged per dst block [128, NBLK] and per row tile [128, NT]
    NT = (N + 127) // 128
    dinv_dst = np.ones((NCORES, 128, NBLK), dtype=np.float32)
    for c in range(NCORES):
        v = dinv[c * NPC:(c + 1) * NPC]
        vp = np.ones(NBLK * 128, dtype=np.float32)
        vp[:NPC] = v
        dinv_dst[c] = vp.reshape(NBLK, 128).T
    dr = np.ones(NT * 128, dtype=np.float32)
    dr[:N] = dinv
    dinv_rows = dr.reshape(NT, 128).T.copy()

    return dict(srcs=srcs, dstl=dstl, kb=kb, col_off=col_off,
                dinv_dst=dinv_dst, dinv_rows=dinv_rows, ktot=ktot)


def _build_layer(kb, col_off, reps=1):
    """One GCN layer: tbl [N,128] -> shard out [NPC,128]."""
    NT = (N + 127) // 128
    nc = bacc.Bacc("TRN2", target_bir_lowering=False, debug=False,
                   num_devices=NCORES)
    tbl = nc.dram_tensor("tbl", [N, P], f32, kind="ExternalInput").ap()
    W = nc.dram_tensor("W", [P, P], f32, kind="ExternalInput").ap()
    brep = nc.dram_tensor("brep", [P, P], f32, kind="ExternalInput").ap()
    thr = nc.dram_tensor("thr", [P, 1], f32, kind="ExternalInput").ap()
    iota = nc.dram_tensor("iota", [P, P], f32, kind="ExternalInput").ap()
    ktot = int(kb.sum())
    srcs = nc.dram_tensor("srcs", [P, ktot], i32, kind="ExternalInput").ap()
    dstl = nc.dram_tensor("dstl", [P, ktot], f32, kind="ExternalInput").ap()
    dinv_dst = nc.dram_tensor("dinv_dst", [P, NBLK], f32,
                              kind="ExternalInput").ap()
    dinv_rows = nc.dram_tensor("dinv_rows", [P, NT], f32,
                               kind="ExternalInput").ap()
    out = nc.dram_tensor("out", [NPC, P], f32, kind="ExternalOutput").ap()

    with tile.TileContext(nc) as tc:
        with (tc.tile_pool(name="const", bufs=1) as cpool,
              tc.tile_pool(name="xin", bufs=8) as xpool,
              tc.tile_pool(name="hs", bufs=8) as hpool,
              tc.tile_pool(name="g", bufs=48) as gpool,
              tc.tile_pool(name="m", bufs=16) as mpool,
              tc.tile_pool(name="ob", bufs=8) as opool,
              tc.tile_pool(name="dram", bufs=1, space="DRAM") as dpool):
            htab = dpool.tile([N, P], f32, name="htab")
            W_t = cpool.tile([P, P], f32, name="W_t")
            nc.sync.dma_start(out=W_t[:], in_=W[:])
            brep_t = cpool.tile([P, P], f32, name="brep_t")
            nc.sync.dma_start(out=brep_t[:], in_=brep[:])
            thr_t = cpool.tile([P, 1], f32, name="thr_t")
            nc.sync.dma_start(out=thr_t[:], in_=thr[:])
            iota_t = cpool.tile([P, P], f32, name="iota_t")
            nc.sync.dma_start(out=iota_t[:], in_=iota[:])
            srcs_t = cpool.tile([P, ktot], i32, name="srcs_t")
            nc.sync.dma_start(out=srcs_t[:], in_=srcs[:])
            dstl_t = cpool.tile([P, ktot], f32, name="dstl_t")
            nc.sync.dma_start(out=dstl_t[:], in_=dstl[:])
            dd_t = cpool.tile([P, NBLK], f32, name="dd_t")
            nc.sync.dma_start(out=dd_t[:], in_=dinv_dst[:])
            dr_t = cpool.tile([P, NT], f32, name="dr_t")
            nc.sync.dma_start(out=dr_t[:], in_=dinv_rows[:])
            ident = cpool.tile([P, P], f32, name="ident")
            make_identity(nc, ident[:])

            rep_cm = tc.For_i(0, reps, 1) if reps > 1 else None
            if rep_cm is not None:
                rep_cm.__enter__()
            # dense: htab = dinv * (tbl @ W)
            with tc.tile_pool(name="psA", bufs=2, space="PSUM") as psA:
                for t in range(NT):
                    r0 = t * 128
                    rows = min(128, N - r0)
                    xt_ = xpool.tile([P, P], f32, name="xt", tag="xt")
                    nc.sync.dma_start(out=xt_[:rows, :],
                                      in_=tbl[r0:r0+rows, :])
                    psT = psA.tile([P, P], f32, name="psT", tag="psT")
                    nc.tensor.transpose(out=psT[:, :rows], in_=xt_[:rows, :],
                                        identity=ident[:rows, :rows])
                    xT = xpool.tile([P, P], f32, name="xT", tag="xT")
                    nc.vector.tensor_copy(out=xT[:, :rows], in_=psT[:, :rows])
                    ps = psA.tile([P, P], f32, name="psA", tag="psA")
                    nc.tensor.matmul(ps[:rows, :], lhsT=xT[:, :rows],
                                     rhs=W_t[:], start=True, stop=True)
                    hs = hpool.tile([P, P], f32, name="hs", tag="hs")
                    nc.vector.tensor_scalar(
                        out=hs[:rows, :], in0=ps[:rows, :],
                        scalar1=dr_t[:rows, t:t+1], scalar2=None,
                        op0=mybir.AluOpType.mult)
                    nc.sync.dma_start(out=htab[r0:r0+rows, :],
                                      in_=hs[:rows, :])

            # aggregation: interleave G dst blocks round-robin so many
            # independent gather->matmul chains are in flight at once
            G = 8
            with tc.tile_pool(name="psB", bufs=1, space="PSUM") as psB:
                for b0 in range(0, NBLK, G):
                    blocks = list(range(b0, min(b0 + G, NBLK)))
                    pstiles = {}
                    for i, b in enumerate(blocks):
                        pstiles[b] = psB.tile([P, P], f32, name="psB",
                                              tag=f"psB{i}")
                    kmax = max(int(kb[b]) for b in blocks)
                    for k in range(kmax):
                        for b in blocks:
                            kbb = int(kb[b])
                            if k >= kbb:
                                continue
                            col = int(col_off[b]) + k
                            g = gpool.tile([P, P], f32, name="g", tag="g")
                            nc.gpsimd.indirect_dma_start(
                                out=g[:], out_offset=None, in_=htab[:],
                                in_offset=bass.IndirectOffsetOnAxis(
                                    ap=srcs_t[:, col:col+1], axis=0))
                            M = mpool.tile([P, P], f32, name="M", tag="M")
                            nc.vector.tensor_scalar(
                                out=M[:], in0=iota_t[:],
                                scalar1=dstl_t[:, col:col+1], scalar2=None,
                                op0=mybir.AluOpType.is_equal)
                            nc.tensor.matmul(pstiles[b][:], lhsT=M[:],
                                             rhs=g[:], start=(k == 0),
                                             stop=(k == kbb - 1))
                    for b in blocks:
                        rows = min(128, NPC - b * 128)
                        ob = opool.tile([P, P], f32, name="ob", tag="ob")
                        nc.vector.tensor_scalar(
                            out=ob[:], in0=pstiles[b][:],
                            scalar1=dd_t[:, b:b+1], scalar2=None,
                            op0=mybir.AluOpType.mult)
                        nc.vector.tensor_tensor(
                            out=ob[:], in0=ob[:], in1=brep_t[:],
                            op=mybir.AluOpType.add)
                        nc.vector.tensor_scalar(
                            out=ob[:], in0=ob[:], scalar1=thr_t[:, :1],
                            scalar2=None, op0=mybir.AluOpType.max)
                        nc.sync.dma_start(out=out[b*128:b*128+rows, :],
                                          in_=ob[:rows, :])
            if rep_cm is not None:
                rep_cm.__exit__(None, None, None)
    nc.compile()
    return nc


def _build_decode(reps=1):
    """Decode: out[p, c] = sum_f z[i[p,c], f] * z[j[p,c], f]."""
    CC = EL // NCORES // 128  # 1024 chunks
    F = OUT
    nc = bacc.Bacc("TRN2", target_bir_lowering=False, debug=False,
                   num_devices=NCORES)
    z = nc.dram_tensor("z", [N, F], f32, kind="ExternalInput").ap()
    ii = nc.dram_tensor("ii", [P, CC], i32, kind="ExternalInput").ap()
    jj = nc.dram_tensor("jj", [P, CC], i32, kind="ExternalInput").ap()
    o = nc.dram_tensor("o", [P, CC], f32, kind="ExternalOutput").ap()
    GD = 24
    with tile.TileContext(nc) as tc:
        with tc.tile_pool(name="sbuf", bufs=1) as pool:
            ii_t = pool.tile([P, CC], i32, name="ii_t")
            nc.sync.dma_start(out=ii_t[:], in_=ii[:])
            jj_t = pool.tile([P, CC], i32, name="jj_t")
            nc.sync.dma_start(out=jj_t[:], in_=jj[:])
            oc = pool.tile([P, CC], f32, name="oc")
            rep_cm = tc.For_i(0, reps, 1) if reps > 1 else None
            if rep_cm is not None:
                rep_cm.__enter__()
            for c in range(CC):
                gi_ = pool.tile([P, F], f32, name=f"gi{c%GD}", tag=f"gi{c%GD}")
                nc.gpsimd.indirect_dma_start(
                    out=gi_[:], out_offset=None, in_=z[:],
                    in_offset=bass.IndirectOffsetOnAxis(
                        ap=ii_t[:, c:c+1], axis=0))
                gj_ = pool.tile([P, F], f32, name=f"gj{c%GD}", tag=f"gj{c%GD}")
                nc.gpsimd.indirect_dma_start(
                    out=gj_[:], out_offset=None, in_=z[:],
                    in_offset=bass.IndirectOffsetOnAxis(
                        ap=jj_t[:, c:c+1], axis=0))
                pr = pool.tile([P, F], f32, name=f"pr{c%8}", tag=f"pr{c%8}")
                nc.vector.tensor_tensor(out=pr[:], in0=gi_[:], in1=gj_[:],
                                        op=mybir.AluOpType.mult)
                nc.vector.tensor_reduce(
                    out=oc[:, c:c+1], in_=pr[:], axis=mybir.AxisListType.X,
                    op=mybir.AluOpType.add)
            if rep_cm is not None:
                rep_cm.__exit__(None, None, None)
            nc.sync.dma_start(out=o[:], in_=oc[:])
    nc.compile()
    return nc


def _get_programs(meta):
    key = ("progs", meta["ktot"], tuple(meta["kb"].tolist()))
    if key not in _prog_cache:
        _prog_cache[key] = (_build_layer(meta["kb"], meta["col_off"]),
                            _build_decode())
    return _prog_cache[key]


def kernel(x, W1, b1, W2, b2, edge_index, edge_label_idx):
    x = np.asarray(x, dtype=np.float32)
    W1 = np.asarray(W1, dtype=np.float32)
    b1 = np.asarray(b1, dtype=np.float32)
    W2 = np.asarray(W2, dtype=np.float32)
    b2 = np.asarray(b2, dtype=np.float32)
    eidx = np.asarray(edge_index)
    eli = np.asarray(edge_label_idx)

    meta = _prep(eidx)
    nc_layer, nc_dec = _get_programs(meta)

    iota = np.broadcast_to(np.arange(P, dtype=np.float32)[None, :],
                           (P, P)).copy()
    W2p = np.zeros((P, P), np.float32)
    W2p[:, :OUT] = W2
    b1rep = np.broadcast_to(b1[None, :], (P, P)).copy().astype(np.float32)
    b2rep = np.zeros((P, P), np.float32)
    b2rep[:, :OUT] = b2[None, :]
    thr_relu = np.zeros((P, 1), np.float32)
    thr_id = np.full((P, 1), -1e30, np.float32)

    def layer_maps(tblv, Wv, brv, thv):
        return [
            {"tbl": tblv, "W": Wv, "brep": brv, "thr": thv, "iota": iota,
             "srcs": meta["srcs"][c], "dstl": meta["dstl"][c],
             "dinv_dst": meta["dinv_dst"][c], "dinv_rows": meta["dinv_rows"]}
            for c in range(NCORES)
        ]

    core_ids = list(range(NCORES))
    # layer 1
    res1 = run_bass_kernel_spmd(
        nc_layer, layer_maps(x, W1, b1rep, thr_relu), core_ids)
    h1 = np.concatenate([res1.results[c]["out"] for c in range(NCORES)],
                        axis=0)
    # layer 2 (padded to 128 feats; cols 64.. are exactly 0)
    res2 = run_bass_kernel_spmd(
        nc_layer, layer_maps(h1, W2p, b2rep, thr_id), core_ids)
    zfull = np.concatenate([res2.results[c]["out"] for c in range(NCORES)],
                           axis=0)
    # decode
    z64 = np.ascontiguousarray(zfull[:, :OUT])
    PPC = EL // NCORES
    CC = PPC // 128
    dec_maps = []
    for c in range(NCORES):
        i0 = np.asarray(eli[0][c*PPC:(c+1)*PPC], dtype=np.int32)
        j0 = np.asarray(eli[1][c*PPC:(c+1)*PPC], dtype=np.int32)
        dec_maps.append({"z": z64,
                         "ii": i0.reshape(CC, 128).T.copy(),
                         "jj": j0.reshape(CC, 128).T.copy()})
    res3 = run_bass_kernel_spmd(nc_dec, dec_maps, core_ids)
    out = np.concatenate(
        [res3.results[c]["o"].T.reshape(-1) for c in range(NCORES)])
    return out.astype(np.float32)



# revision 4
# speedup vs baseline: 1.7004x; 1.7004x over previous
"""GCN link-prediction kernel for 8 Trainium2 NeuronCores (v2).

Strategy:
  - dst-node sharding across 8 cores (12500 nodes each); each core processes
    edges whose dst is in its shard (+ self loops).
  - GCN sym-norm factorizes: out[d] = dinv[d] * sum dinv[s]*h[s]; the dinv
    pre-scale is folded into the host-side input prep (x' = dinv*x), the
    post-scale runs per dst block on DVE.
  - Dense (h = x'@W): host supplies x'^T fp16 [128, N]; big chunked loads,
    direct matmul (lhsT = x'^T slice), ACT-engine cast to fp16 table in DRAM.
  - Aggregation: edges sorted by (psum-stripe of 8 dst blocks, src range of
    32768, dst block); per (stripe, range) ONE big dma_gather (int16 local
    indices) pulls h rows; per 128-edge column a one-hot matrix (iota
    is_equal dstl) built on DVE selects/accumulates rows into the block's
    PSUM tile via PE matmul (fp16, fp32 accum).
  - Both layers share one compiled program (W2 zero-padded; relu vs identity
    via max-threshold).
  - Decode: pairs sorted by (range(i), range(j)); two big gather streams of
    z rows (fp32, 256B); DVE multiply + row-reduce per 128-pair column.
Host does index prep (sorting, padding, int16 wrap), input pre-scaling /
transposes, and inter-program stitching.
"""
import numpy as np

import concourse.bass as bass
import concourse.bacc as bacc
import concourse.mybir as mybir
import concourse.tile as tile
from concourse.bass_utils import run_bass_kernel_spmd

f32 = mybir.dt.float32
f16 = mybir.dt.float16
i16 = mybir.dt.int16

N = 100000
E = 1600000
EL = 1048576
IN = 128
HID = 128
OUT = 64
NCORES = 8
NPC = N // NCORES           # 12500 nodes per core
NBLK = (NPC + 127) // 128   # 98 dst blocks per core
P = 128
RS = 32768                  # src range size (int16 gather indices)
NR = (N + RS - 1) // RS     # 4 ranges
SB = 8                      # dst blocks per PSUM stripe
NST = (NBLK + SB - 1) // SB # 13 stripes
GC = 8                      # gather columns per dma_gather (1024-idx cap)
GC2 = 8                     # gather columns per dma_gather (decode)
PPC = EL // NCORES          # 131072 pairs per core

_prog_cache = {}


def _wrap_idx(vals):
    """int16 value list -> [128, n/16] tile (i at [i%16, i//16], replicated
    across the 8 groups of 16 partitions)."""
    w = vals.reshape(-1, 16).T  # [16, n/16]
    return np.tile(w, (8, 1)).copy()


def _prep(edge_index):
    src = np.asarray(edge_index[0], dtype=np.int64)
    dst = np.asarray(edge_index[1], dtype=np.int64)
    deg = np.bincount(dst, minlength=N).astype(np.float64) + 1.0
    dinv = (1.0 / np.sqrt(deg)).astype(np.float32)

    NSEG = NST * NR * NBLK
    per_core = []
    counts = np.zeros((NCORES, NSEG), dtype=np.int64)
    for c in range(NCORES):
        m = (dst // NPC) == c
        s = src[m]
        dl = dst[m] - c * NPC
        loop = np.arange(NPC, dtype=np.int64)
        s = np.concatenate([s, loop + c * NPC])
        dl = np.concatenate([dl, loop])
        blk = dl >> 7
        r = s >> 15
        st = blk // SB
        key = (st * NR + r) * NBLK + blk
        order = np.argsort(key, kind="stable")
        s, dl, key = s[order], dl[order], key[order]
        per_core.append((s, dl, key))
        counts[c] = np.bincount(key, minlength=NSEG)

    seg_cols = (counts.max(axis=0) + 127) // 128  # common layout
    active = np.nonzero(seg_cols)[0]
    seg_cols_a = seg_cols[active]
    seg_coloff = np.concatenate([[0], np.cumsum(seg_cols_a)[:-1]])
    ncolt = int(seg_cols_a.sum())
    total = ncolt * 128
    seg_off_of = np.full(NSEG, -1, dtype=np.int64)
    seg_off_of[active] = seg_coloff * 128

    seg_blk = active % NBLK
    seg_r = (active // NBLK) % NR
    seg_st = active // (NBLK * NR)
    colblk = np.repeat(seg_blk, seg_cols_a)

    first_col = {}
    last_col = {}
    for j, b in enumerate(colblk):
        b = int(b)
        if b not in first_col:
            first_col[b] = j
        last_col[b] = j

    # gather instructions: contiguous (st, r) runs chunked by GC columns
    instrs = []  # (range_base, range_len, col0, ncols)
    i = 0
    nact = len(active)
    while i < nact:
        j = i
        while (j < nact and seg_st[j] == seg_st[i] and seg_r[j] == seg_r[i]):
            j += 1
        c0 = int(seg_coloff[i])
        c1 = int(seg_coloff[j - 1] + seg_cols_a[j - 1])
        base = int(seg_r[i]) * RS
        rlen = min(RS, N - base)
        for cc in range(c0, c1, GC):
            instrs.append((base, rlen, cc, min(GC, c1 - cc)))
        i = j

    idxts = np.zeros((NCORES, 128, total // 16), np.int16)
    dstlts = np.full((NCORES, 128, ncolt), 999.0, np.float32)
    for c in range(NCORES):
        s, dl, key = per_core[c]
        n = s.shape[0]
        uk, seg_start, seg_cnt = np.unique(key, return_index=True,
                                           return_counts=True)
        seg_of = np.repeat(np.arange(len(uk)), seg_cnt)
        within = np.arange(n) - seg_start[seg_of]
        pos = seg_off_of[uk[seg_of]] + within
        src_loc = np.zeros(total, np.int16)
        src_loc[pos] = (s - RS * (s >> 15)).astype(np.int16)
        dstl = np.full(total, 999.0, np.float32)
        dstl[pos] = (dl & 127).astype(np.float32)
        idxts[c] = _wrap_idx(src_loc)
        dstlts[c] = dstl.reshape(-1, 128).T

    dinv_dst = np.ones((NCORES, 128, NBLK), dtype=np.float32)
    for c in range(NCORES):
        v = dinv[c * NPC:(c + 1) * NPC]
        vp = np.ones(NBLK * 128, dtype=np.float32)
        vp[:NPC] = v
        dinv_dst[c] = vp.reshape(NBLK, 128).T

    struct = (ncolt, tuple(colblk.tolist()), tuple(instrs))
    return dict(idxts=idxts, dstlts=dstlts, dinv=dinv, dinv_dst=dinv_dst,
                ncolt=ncolt, colblk=colblk, first_col=first_col,
                last_col=last_col, instrs=instrs, struct=struct)


def _prep_decode(edge_label_idx):
    eli = np.asarray(edge_label_idx)
    NSEG = NR * NR
    per_core = []
    counts = np.zeros((NCORES, NSEG), dtype=np.int64)
    for c in range(NCORES):
        i0 = np.asarray(eli[0][c * PPC:(c + 1) * PPC], dtype=np.int64)
        j0 = np.asarray(eli[1][c * PPC:(c + 1) * PPC], dtype=np.int64)
        key = (i0 >> 15) * NR + (j0 >> 15)
        order = np.argsort(key, kind="stable")
        per_core.append((i0[order], j0[order], key[order], order))
        counts[c] = np.bincount(key, minlength=NSEG)

    seg_cols = (counts.max(axis=0) + 127) // 128
    active = np.nonzero(seg_cols)[0]
    seg_cols_a = seg_cols[active]
    seg_coloff = np.concatenate([[0], np.cumsum(seg_cols_a)[:-1]])
    ncc = int(seg_cols_a.sum())
    total = ncc * 128
    seg_off_of = np.full(NSEG, -1, dtype=np.int64)
    seg_off_of[active] = seg_coloff * 128
    seg_ri = active // NR
    seg_rj = active % NR

    # i-stream instructions: contiguous runs of same ri, chunked by GC2
    iinstr = []
    i = 0
    nact = len(active)
    while i < nact:
        j = i
        while j < nact and seg_ri[j] == seg_ri[i]:
            j += 1
        c0 = int(seg_coloff[i])
        c1 = int(seg_coloff[j - 1] + seg_cols_a[j - 1])
        base = int(seg_ri[i]) * RS
        rlen = min(RS, N - base)
        for cc in range(c0, c1, GC2):
            iinstr.append((base, rlen, cc, min(GC2, c1 - cc)))
        i = j
    # j-stream instructions: per segment, chunked by GC2
    jinstr = []
    for k in range(nact):
        c0 = int(seg_coloff[k])
        c1 = c0 + int(seg_cols_a[k])
        base = int(seg_rj[k]) * RS
        rlen = min(RS, N - base)
        for cc in range(c0, c1, GC2):
            jinstr.append((base, rlen, cc, min(GC2, c1 - cc)))

    iits = np.zeros((NCORES, 128, total // 16), np.int16)
    jjts = np.zeros((NCORES, 128, total // 16), np.int16)
    restore = np.zeros((NCORES, PPC), np.int64)  # padded pos of sorted pair
    origs = np.zeros((NCORES, PPC), np.int64)
    for c in range(NCORES):
        i0, j0, key, order = per_core[c]
        n = i0.shape[0]
        uk, seg_start, seg_cnt = np.unique(key, return_index=True,
                                           return_counts=True)
        seg_of = np.repeat(np.arange(len(uk)), seg_cnt)
        within = np.arange(n) - seg_start[seg_of]
        pos = seg_off_of[uk[seg_of]] + within
        ii = np.zeros(total, np.int16)
        jj = np.zeros(total, np.int16)
        ii[pos] = (i0 - RS * (i0 >> 15)).astype(np.int16)
        jj[pos] = (j0 - RS * (j0 >> 15)).astype(np.int16)
        iits[c] = _wrap_idx(ii)
        jjts[c] = _wrap_idx(jj)
        restore[c] = pos
        origs[c] = order

    struct = (ncc, tuple(iinstr), tuple(jinstr))
    return dict(iits=iits, jjts=jjts, restore=restore, origs=origs,
                ncc=ncc, iinstr=iinstr, jinstr=jinstr, struct=struct)


def _build_layer(meta, reps=1):
    """One GCN layer: xT fp16 [128, N] (pre-scaled) -> out shard [NPC, 128] f32."""
    ncolt = meta["ncolt"]
    colblk = meta["colblk"]
    first_col = meta["first_col"]
    last_col = meta["last_col"]
    instrs = meta["instrs"]
    IW = ncolt * 128 // 16
    NT = (N + 127) // 128      # 782 dense tiles
    QUAD = 4                   # dense tiles per table-write DMA
    CHW = 4096                 # dense chunk width (nodes)

    nc = bacc.Bacc("TRN2", target_bir_lowering=False, debug=False,
                   num_devices=NCORES, num_swdge_queues=4)
    xT = nc.dram_tensor("xT", [P, N], f16, kind="ExternalInput").ap()
    W = nc.dram_tensor("W", [P, P], f16, kind="ExternalInput").ap()
    brep = nc.dram_tensor("brep", [P, P], f32, kind="ExternalInput").ap()
    thr = nc.dram_tensor("thr", [P, 1], f32, kind="ExternalInput").ap()
    iota = nc.dram_tensor("iota", [P, P], f16, kind="ExternalInput").ap()
    idxs = nc.dram_tensor("idxs", [P, IW], i16, kind="ExternalInput").ap()
    dstl = nc.dram_tensor("dstl", [P, ncolt], f32, kind="ExternalInput").ap()
    dinv_dst = nc.dram_tensor("dinv_dst", [P, NBLK], f32,
                              kind="ExternalInput").ap()
    out = nc.dram_tensor("out", [NPC, P], f32, kind="ExternalOutput").ap()

    with tile.TileContext(nc) as tc:
        with (tc.tile_pool(name="const", bufs=1) as cpool,
              tc.tile_pool(name="xin", bufs=2) as xpool,
              tc.tile_pool(name="hs", bufs=3) as hpool,
              tc.tile_pool(name="g", bufs=6) as gpool,
              tc.tile_pool(name="m", bufs=8) as mpool,
              tc.tile_pool(name="ob", bufs=4) as opool,
              tc.tile_pool(name="dram", bufs=1, space="DRAM") as dpool):
            htab = dpool.tile([N, P], f16, name="htab")
            W_t = cpool.tile([P, P], f16, name="W_t")
            nc.sync.dma_start(out=W_t[:], in_=W[:])
            brep_t = cpool.tile([P, P], f32, name="brep_t")
            nc.sync.dma_start(out=brep_t[:], in_=brep[:])
            thr_t = cpool.tile([P, 1], f32, name="thr_t")
            nc.sync.dma_start(out=thr_t[:], in_=thr[:])
            iota_t = cpool.tile([P, P], f16, name="iota_t")
            nc.sync.dma_start(out=iota_t[:], in_=iota[:])
            idxs_t = cpool.tile([P, IW], i16, name="idxs_t")
            nc.sync.dma_start(out=idxs_t[:], in_=idxs[:])
            dstl_t = cpool.tile([P, ncolt], f32, name="dstl_t")
            nc.sync.dma_start(out=dstl_t[:], in_=dstl[:])
            dd_t = cpool.tile([P, NBLK], f32, name="dd_t")
            nc.sync.dma_start(out=dd_t[:], in_=dinv_dst[:])

            rep_cm = tc.For_i(0, reps, 1) if reps > 1 else None
            if rep_cm is not None:
                rep_cm.__enter__()

            # ---- dense: htab = fp16(xT^T @ W) ----
            with tc.tile_pool(name="psD", bufs=2, space="PSUM") as psD:
                for off in range(0, N, CHW):
                    w = min(CHW, N - off)
                    xc = xpool.tile([P, CHW], f16, name="xc", tag="xc")
                    nc.sync.dma_start(out=xc[:, :w], in_=xT[:, off:off + w])
                    for t0 in range(0, w, QUAD * 128):
                        nt = min(QUAD * 128, w - t0)
                        nq = (nt + 127) // 128
                        hq = hpool.tile([P, QUAD, P], f16, name="hq", tag="hq")
                        for q in range(nq):
                            rows = min(128, nt - q * 128)
                            ps = psD.tile([P, P], f32, name="ps", tag="ps")
                            nc.tensor.matmul(
                                ps[:rows, :],
                                lhsT=xc[:, t0 + q * 128:t0 + q * 128 + rows],
                                rhs=W_t[:], start=True, stop=True)
                            nc.scalar.copy(out=hq[:rows, q, :],
                                           in_=ps[:rows, :])
                        r0 = off + t0
                        rr = min(QUAD * 128, N - r0)
                        if rr % 128 == 0:
                            nc.sync.dma_start(
                                out=htab[r0:r0 + rr, :].rearrange(
                                    "(q p) f -> p q f", p=P),
                            in_=hq[:, :nq, :])
                        else:
                            for q in range(nq):
                                rows = min(128, rr - q * 128)
                                nc.sync.dma_start(
                                    out=htab[r0 + q * 128:
                                             r0 + q * 128 + rows, :],
                                    in_=hq[:rows, q, :])

            # ---- aggregation ----
            with tc.tile_pool(name="psA", bufs=1, space="PSUM") as psA:
                for st in range(NST):
                    blocks = list(range(st * SB, min(st * SB + SB, NBLK)))
                    pstiles = {}
                    for i, b in enumerate(blocks):
                        pstiles[b] = psA.tile([P, P], f32, name="psA",
                                              tag=f"psA{i}")
                    for gi, (base, rlen, c0, ncols) in enumerate(instrs):
                        if colblk[c0] // SB != st:
                            continue
                        gt = gpool.tile([P, GC, P], f16, name="gt", tag="gt")
                        nc.gpsimd.dma_gather(
                            gt[:, :ncols, :], htab[base:base + rlen, :],
                            idxs_t[:, c0 * 8:(c0 + ncols) * 8],
                            ncols * 128, ncols * 128, P, queue_num=gi % 4)
                        for k in range(ncols):
                            j = c0 + k
                            b = int(colblk[j])
                            M = mpool.tile([P, P], f16, name="M", tag="M")
                            nc.vector.tensor_scalar(
                                out=M[:], in0=iota_t[:],
                                scalar1=dstl_t[:, j:j + 1], scalar2=None,
                                op0=mybir.AluOpType.is_equal)
                            nc.tensor.matmul(pstiles[b][:], lhsT=M[:],
                                             rhs=gt[:, k, :],
                                             start=(j == first_col[b]),
                                             stop=(j == last_col[b]))
                    for b in blocks:
                        rows = min(128, NPC - b * 128)
                        ob = opool.tile([P, P], f32, name="ob", tag="ob")
                        nc.vector.tensor_scalar(
                            out=ob[:], in0=pstiles[b][:],
                            scalar1=dd_t[:, b:b + 1], scalar2=None,
                            op0=mybir.AluOpType.mult)
                        nc.vector.tensor_tensor(
                            out=ob[:], in0=ob[:], in1=brep_t[:],
                            op=mybir.AluOpType.add)
                        nc.vector.tensor_scalar(
                            out=ob[:], in0=ob[:], scalar1=thr_t[:, :1],
                            scalar2=None, op0=mybir.AluOpType.max)
                        nc.sync.dma_start(out=out[b * 128:b * 128 + rows, :],
                                          in_=ob[:rows, :])
            if rep_cm is not None:
                rep_cm.__exit__(None, None, None)
    nc.compile()
    return nc


def _build_decode(dmeta, reps=1):
    """Decode: o[p, c] = sum_f z[ii[p,c], f] * z[jj[p,c], f]."""
    ncc = dmeta["ncc"]
    iinstr = dmeta["iinstr"]
    jinstr = dmeta["jinstr"]
    IW = ncc * 128 // 16
    F = OUT

    nc = bacc.Bacc("TRN2", target_bir_lowering=False, debug=False,
                   num_devices=NCORES, num_swdge_queues=4)
    z = nc.dram_tensor("z", [N, F], f32, kind="ExternalInput").ap()
    ii = nc.dram_tensor("ii", [P, IW], i16, kind="ExternalInput").ap()
    jj = nc.dram_tensor("jj", [P, IW], i16, kind="ExternalInput").ap()
    o = nc.dram_tensor("o", [P, ncc], f32, kind="ExternalOutput").ap()

    with tile.TileContext(nc) as tc:
        with (tc.tile_pool(name="c", bufs=1) as cpool,
              tc.tile_pool(name="gi", bufs=6) as gipool,
              tc.tile_pool(name="gj", bufs=6) as gjpool,
              tc.tile_pool(name="pr", bufs=8) as prpool):
            ii_t = cpool.tile([P, IW], i16, name="ii_t")
            nc.sync.dma_start(out=ii_t[:], in_=ii[:])
            jj_t = cpool.tile([P, IW], i16, name="jj_t")
            nc.sync.dma_start(out=jj_t[:], in_=jj[:])
            oc = cpool.tile([P, ncc], f32, name="oc")

            rep_cm = tc.For_i(0, reps, 1) if reps > 1 else None
            if rep_cm is not None:
                rep_cm.__enter__()
            # merge the two instruction streams in column order
            events = ([("i", t) for t in iinstr] + [("j", t) for t in jinstr])
            events.sort(key=lambda e: (e[1][2], 0 if e[0] == "i" else 1))
            qn = 0
            itiles = {}  # col -> (tile, col0)
            jtiles = {}
            done = set()
            for (kind, (base, rlen, c0, ncols)) in events:
                if kind == "i":
                    gt = gipool.tile([P, GC2, F], f32, name="git", tag="git")
                    nc.gpsimd.dma_gather(
                        gt[:, :ncols, :], z[base:base + rlen, :],
                        ii_t[:, c0 * 8:(c0 + ncols) * 8],
                        ncols * 128, ncols * 128, F, queue_num=qn % 4)
                    qn += 1
                    for k in range(ncols):
                        itiles[c0 + k] = (gt, k)
                else:
                    gt = gjpool.tile([P, GC2, F], f32, name="gjt", tag="gjt")
                    nc.gpsimd.dma_gather(
                        gt[:, :ncols, :], z[base:base + rlen, :],
                        jj_t[:, c0 * 8:(c0 + ncols) * 8],
                        ncols * 128, ncols * 128, F, queue_num=qn % 4)
                    qn += 1
                    for k in range(ncols):
                        jtiles[c0 + k] = (gt, k)
                ready = [c for c in itiles if c in jtiles and c not in done]
                for c in sorted(ready):
                    (git, ki) = itiles[c]
                    (gjt, kj) = jtiles[c]
                    pr = prpool.tile([P, F], f32, name="pr", tag="pr")
                    nc.vector.tensor_tensor(out=pr[:], in0=git[:, ki, :],
                                            in1=gjt[:, kj, :],
                                            op=mybir.AluOpType.mult)
                    nc.vector.tensor_reduce(
                        out=oc[:, c:c + 1], in_=pr[:],
                        axis=mybir.AxisListType.X, op=mybir.AluOpType.add)
                    done.add(c)
            if rep_cm is not None:
                rep_cm.__exit__(None, None, None)
            nc.sync.dma_start(out=o[:], in_=oc[:])
    nc.compile()
    return nc


def _get_programs(meta, dmeta):
    key = ("progs", meta["struct"], dmeta["struct"])
    if key not in _prog_cache:
        _prog_cache[key] = (_build_layer(meta), _build_decode(dmeta))
    return _prog_cache[key]


def _layer_maps(meta, xTv, Wv, brv, thv):
    iota = np.broadcast_to(np.arange(P, dtype=np.float16)[None, :],
                           (P, P)).copy()
    return [
        {"xT": xTv, "W": Wv, "brep": brv, "thr": thv, "iota": iota,
         "idxs": meta["idxts"][c], "dstl": meta["dstlts"][c],
         "dinv_dst": meta["dinv_dst"][c]}
        for c in range(NCORES)
    ]


def kernel(x, W1, b1, W2, b2, edge_index, edge_label_idx):
    x = np.asarray(x, dtype=np.float32)
    W1 = np.asarray(W1, dtype=np.float32)
    b1 = np.asarray(b1, dtype=np.float32)
    W2 = np.asarray(W2, dtype=np.float32)
    b2 = np.asarray(b2, dtype=np.float32)
    eidx = np.asarray(edge_index)

    meta = _prep(eidx)
    dmeta = _prep_decode(edge_label_idx)
    nc_layer, nc_dec = _get_programs(meta, dmeta)
    dinv = meta["dinv"]

    W1h = W1.astype(np.float16)
    W2h = np.zeros((P, P), np.float16)
    W2h[:, :OUT] = W2
    b1rep = np.broadcast_to(b1[None, :], (P, P)).astype(np.float32).copy()
    b2rep = np.zeros((P, P), np.float32)
    b2rep[:, :OUT] = b2[None, :]
    thr_relu = np.zeros((P, 1), np.float32)
    thr_id = np.full((P, 1), -1e30, np.float32)

    core_ids = list(range(NCORES))
    # layer 1
    x1T = np.ascontiguousarray((x * dinv[:, None]).astype(np.float16).T)
    res1 = run_bass_kernel_spmd(
        nc_layer, _layer_maps(meta, x1T, W1h, b1rep, thr_relu), core_ids)
    h1 = np.concatenate([res1.results[c]["out"] for c in range(NCORES)],
                        axis=0)
    # layer 2
    x2T = np.ascontiguousarray((h1 * dinv[:, None]).astype(np.float16).T)
    res2 = run_bass_kernel_spmd(
        nc_layer, _layer_maps(meta, x2T, W2h, b2rep, thr_id), core_ids)
    zfull = np.concatenate([res2.results[c]["out"] for c in range(NCORES)],
                           axis=0)
    z64 = np.ascontiguousarray(zfull[:, :OUT])
    # decode
    dec_maps = [{"z": z64, "ii": dmeta["iits"][c], "jj": dmeta["jjts"][c]}
                for c in range(NCORES)]
    res3 = run_bass_kernel_spmd(nc_dec, dec_maps, core_ids)
    out = np.empty(EL, np.float32)
    for c in range(NCORES):
        flat = res3.results[c]["o"].T.reshape(-1)
        out[c * PPC + dmeta["origs"][c]] = flat[dmeta["restore"][c]]
    return out.astype(np.float32)


# revision 12
# speedup vs baseline: 2.4209x; 1.4237x over previous
"""GCN link-prediction kernel for 8 Trainium2 NeuronCores (v2).

Strategy:
  - dst-node sharding across 8 cores (12500 nodes each); each core processes
    edges whose dst is in its shard (+ self loops).
  - GCN sym-norm factorizes: out[d] = dinv[d] * sum dinv[s]*h[s]; the dinv
    pre-scale is folded into the host-side input prep (x' = dinv*x), the
    post-scale runs per dst block on DVE.
  - Dense (h = x'@W): host supplies x'^T fp16 [128, N]; big chunked loads,
    direct matmul (lhsT = x'^T slice), ACT-engine cast to fp16 table in DRAM.
  - Aggregation: edges sorted by (psum-stripe of 8 dst blocks, src range of
    32768, dst block); per (stripe, range) ONE big dma_gather (int16 local
    indices) pulls h rows; per 128-edge column a one-hot matrix (iota
    is_equal dstl) built on DVE selects/accumulates rows into the block's
    PSUM tile via PE matmul (fp16, fp32 accum).
  - Both layers share one compiled program (W2 zero-padded; relu vs identity
    via max-threshold).
  - Decode: pairs sorted by (range(i), range(j)); two big gather streams of
    z rows (fp32, 256B); DVE multiply + row-reduce per 128-pair column.
Host does index prep (sorting, padding, int16 wrap), input pre-scaling /
transposes, and inter-program stitching.
"""
import numpy as np

import concourse.bass as bass
import concourse.bacc as bacc
import concourse.mybir as mybir
import concourse.tile as tile
from concourse.bass_utils import run_bass_kernel_spmd

f32 = mybir.dt.float32
f16 = mybir.dt.float16
i16 = mybir.dt.int16

N = 100000
E = 1600000
EL = 1048576
IN = 128
HID = 128
OUT = 64
NCORES = 8
NPC = N // NCORES           # 12500 nodes per core
NBLK = (NPC + 127) // 128   # 98 dst blocks per core
P = 128
RS = 32768                  # src range size (int16 gather indices)
NR = (N + RS - 1) // RS     # 4 ranges
SB = 8                      # dst blocks per PSUM stripe
NST = (NBLK + SB - 1) // SB # 13 stripes
GC = 8                      # gather columns per dma_gather (1024-idx cap)
GC2 = 8                     # gather columns per dma_gather (decode)
PPC = EL // NCORES          # 131072 pairs per core

_prog_cache = {}


def _wrap_idx(vals):
    """int16 value list -> [128, n/16] tile (i at [i%16, i//16], replicated
    across the 8 groups of 16 partitions)."""
    w = vals.reshape(-1, 16).T  # [16, n/16]
    return np.tile(w, (8, 1)).copy()


def _prep(edge_index):
    src = np.asarray(edge_index[0], dtype=np.int64)
    dst = np.asarray(edge_index[1], dtype=np.int64)
    deg = np.bincount(dst, minlength=N).astype(np.float64) + 1.0
    dinv = (1.0 / np.sqrt(deg)).astype(np.float32)

    NSEG = NST * NR * NBLK
    per_core = []
    counts = np.zeros((NCORES, NSEG), dtype=np.int64)
    for c in range(NCORES):
        m = (dst // NPC) == c
        s = src[m]
        dl = dst[m] - c * NPC
        loop = np.arange(NPC, dtype=np.int64)
        s = np.concatenate([s, loop + c * NPC])
        dl = np.concatenate([dl, loop])
        blk = dl >> 7
        r = s >> 15
        st = blk // SB
        key = (st * NR + r) * NBLK + blk
        order = np.argsort(key, kind="stable")
        s, dl, key = s[order], dl[order], key[order]
        per_core.append((s, dl, key))
        counts[c] = np.bincount(key, minlength=NSEG)

    seg_cols = (counts.max(axis=0) + 127) // 128  # common layout
    active = np.nonzero(seg_cols)[0]
    seg_cols_a = seg_cols[active]
    seg_coloff = np.concatenate([[0], np.cumsum(seg_cols_a)[:-1]])
    ncolt = int(seg_cols_a.sum())
    total = ncolt * 128
    seg_off_of = np.full(NSEG, -1, dtype=np.int64)
    seg_off_of[active] = seg_coloff * 128

    seg_blk = active % NBLK
    seg_r = (active // NBLK) % NR
    seg_st = active // (NBLK * NR)
    colblk = np.repeat(seg_blk, seg_cols_a)

    first_col = {}
    last_col = {}
    for j, b in enumerate(colblk):
        b = int(b)
        if b not in first_col:
            first_col[b] = j
        last_col[b] = j

    # gather instructions: contiguous (st, r) runs chunked by GC columns
    instrs = []  # (range_base, range_len, col0, ncols)
    i = 0
    nact = len(active)
    while i < nact:
        j = i
        while (j < nact and seg_st[j] == seg_st[i] and seg_r[j] == seg_r[i]):
            j += 1
        c0 = int(seg_coloff[i])
        c1 = int(seg_coloff[j - 1] + seg_cols_a[j - 1])
        base = int(seg_r[i]) * RS
        rlen = min(RS, N - base)
        for cc in range(c0, c1, GC):
            instrs.append((base, rlen, cc, min(GC, c1 - cc)))
        i = j

    idxts = np.zeros((NCORES, 128, total // 16), np.int16)
    dstlts = np.full((NCORES, 128, ncolt), 999.0, np.float32)
    for c in range(NCORES):
        s, dl, key = per_core[c]
        n = s.shape[0]
        uk, seg_start, seg_cnt = np.unique(key, return_index=True,
                                           return_counts=True)
        seg_of = np.repeat(np.arange(len(uk)), seg_cnt)
        within = np.arange(n) - seg_start[seg_of]
        pos = seg_off_of[uk[seg_of]] + within
        src_loc = np.zeros(total, np.int16)
        src_loc[pos] = (s - RS * (s >> 15)).astype(np.int16)
        dstl = np.full(total, 999.0, np.float32)
        dstl[pos] = (dl & 127).astype(np.float32)
        idxts[c] = _wrap_idx(src_loc)
        dstlts[c] = dstl.reshape(-1, 128).T

    dinv_dst = np.ones((NCORES, 128, NBLK), dtype=np.float32)
    for c in range(NCORES):
        v = dinv[c * NPC:(c + 1) * NPC]
        vp = np.ones(NBLK * 128, dtype=np.float32)
        vp[:NPC] = v
        dinv_dst[c] = vp.reshape(NBLK, 128).T

    struct = (ncolt, tuple(colblk.tolist()), tuple(instrs))
    return dict(idxts=idxts, dstlts=dstlts, dinv=dinv, dinv_dst=dinv_dst,
                ncolt=ncolt, colblk=colblk, first_col=first_col,
                last_col=last_col, instrs=instrs, struct=struct)


def _prep_decode(edge_label_idx):
    eli = np.asarray(edge_label_idx)
    NSEG = NR * NR
    per_core = []
    counts = np.zeros((NCORES, NSEG), dtype=np.int64)
    for c in range(NCORES):
        i0 = np.asarray(eli[0][c * PPC:(c + 1) * PPC], dtype=np.int64)
        j0 = np.asarray(eli[1][c * PPC:(c + 1) * PPC], dtype=np.int64)
        key = (i0 >> 15) * NR + (j0 >> 15)
        order = np.argsort(key, kind="stable")
        per_core.append((i0[order], j0[order], key[order], order))
        counts[c] = np.bincount(key, minlength=NSEG)

    seg_cols = (counts.max(axis=0) + 127) // 128
    active = np.nonzero(seg_cols)[0]
    seg_cols_a = seg_cols[active]
    seg_coloff = np.concatenate([[0], np.cumsum(seg_cols_a)[:-1]])
    ncc = int(seg_cols_a.sum())
    total = ncc * 128
    seg_off_of = np.full(NSEG, -1, dtype=np.int64)
    seg_off_of[active] = seg_coloff * 128
    seg_ri = active // NR
    seg_rj = active % NR

    # i-stream instructions: contiguous runs of same ri, chunked by GC2
    iinstr = []
    i = 0
    nact = len(active)
    while i < nact:
        j = i
        while j < nact and seg_ri[j] == seg_ri[i]:
            j += 1
        c0 = int(seg_coloff[i])
        c1 = int(seg_coloff[j - 1] + seg_cols_a[j - 1])
        base = int(seg_ri[i]) * RS
        rlen = min(RS, N - base)
        for cc in range(c0, c1, GC2):
            iinstr.append((base, rlen, cc, min(GC2, c1 - cc)))
        i = j
    # j-stream instructions: per segment, chunked by GC2
    jinstr = []
    for k in range(nact):
        c0 = int(seg_coloff[k])
        c1 = c0 + int(seg_cols_a[k])
        base = int(seg_rj[k]) * RS
        rlen = min(RS, N - base)
        for cc in range(c0, c1, GC2):
            jinstr.append((base, rlen, cc, min(GC2, c1 - cc)))

    iits = np.zeros((NCORES, 128, total // 16), np.int16)
    jjts = np.zeros((NCORES, 128, total // 16), np.int16)
    restore = np.zeros((NCORES, PPC), np.int64)  # padded pos of sorted pair
    origs = np.zeros((NCORES, PPC), np.int64)
    for c in range(NCORES):
        i0, j0, key, order = per_core[c]
        n = i0.shape[0]
        uk, seg_start, seg_cnt = np.unique(key, return_index=True,
                                           return_counts=True)
        seg_of = np.repeat(np.arange(len(uk)), seg_cnt)
        within = np.arange(n) - seg_start[seg_of]
        pos = seg_off_of[uk[seg_of]] + within
        ii = np.zeros(total, np.int16)
        jj = np.zeros(total, np.int16)
        ii[pos] = (i0 - RS * (i0 >> 15)).astype(np.int16)
        jj[pos] = (j0 - RS * (j0 >> 15)).astype(np.int16)
        iits[c] = _wrap_idx(ii)
        jjts[c] = _wrap_idx(jj)
        restore[c] = pos
        origs[c] = order

    struct = (ncc, tuple(iinstr), tuple(jinstr))
    return dict(iits=iits, jjts=jjts, restore=restore, origs=origs,
                ncc=ncc, iinstr=iinstr, jinstr=jinstr, struct=struct)


HTAB_OUT = True


def _build_layer(meta, reps=1, parts="full"):
    """One GCN layer: xT fp16 [128, N] (pre-scaled) -> out shard [NPC, 128] f32."""
    ncolt = meta["ncolt"]
    colblk = meta["colblk"]
    first_col = meta["first_col"]
    last_col = meta["last_col"]
    instrs = meta["instrs"]
    IW = ncolt * 128 // 16
    NT = (N + 127) // 128      # 782 dense tiles
    QUAD = 4                   # dense tiles per table-write DMA
    CHW = 4096                 # dense chunk width (nodes)

    nc = bacc.Bacc("TRN2", target_bir_lowering=False, debug=False,
                   num_devices=NCORES, num_swdge_queues=4)
    xT = nc.dram_tensor("xT", [P, N], f16, kind="ExternalInput").ap()
    W = nc.dram_tensor("W", [P, P], f16, kind="ExternalInput").ap()
    brep = nc.dram_tensor("brep", [P, P], f32, kind="ExternalInput").ap()
    thr = nc.dram_tensor("thr", [P, 1], f32, kind="ExternalInput").ap()
    iota = nc.dram_tensor("iota", [P, P], f16, kind="ExternalInput").ap()
    idxs = nc.dram_tensor("idxs", [P, IW], i16, kind="ExternalInput").ap()
    dstl = nc.dram_tensor("dstl", [P, ncolt], f32, kind="ExternalInput").ap()
    dinv_dst = nc.dram_tensor("dinv_dst", [P, NBLK], f32,
                              kind="ExternalInput").ap()
    out = nc.dram_tensor("out", [NPC, P], f32, kind="ExternalOutput").ap()
    htab_ext = (nc.dram_tensor("htab", [N, P], f16, kind="ExternalOutput").ap()
                if HTAB_OUT else None)

    with tile.TileContext(nc) as tc:
        with (tc.tile_pool(name="const", bufs=1) as cpool,
              tc.tile_pool(name="xin", bufs=2) as xpool,
              tc.tile_pool(name="hs", bufs=4) as hpool,
              tc.tile_pool(name="g", bufs=16) as gpool,
              tc.tile_pool(name="m", bufs=16) as mpool,
              tc.tile_pool(name="ob", bufs=4) as opool,
              tc.tile_pool(name="dram", bufs=1, space="DRAM") as dpool):
            htab = htab_ext if HTAB_OUT else dpool.tile([N, P], f16,
                                                         name="htab")
            W_t = cpool.tile([P, P], f16, name="W_t")
            nc.sync.dma_start(out=W_t[:], in_=W[:])
            brep_t = cpool.tile([P, P], f32, name="brep_t")
            nc.sync.dma_start(out=brep_t[:], in_=brep[:])
            thr_t = cpool.tile([P, 1], f32, name="thr_t")
            nc.sync.dma_start(out=thr_t[:], in_=thr[:])
            iota_t = cpool.tile([P, P], f16, name="iota_t")
            nc.sync.dma_start(out=iota_t[:], in_=iota[:])
            idxs_t = cpool.tile([P, IW], i16, name="idxs_t")
            nc.sync.dma_start(out=idxs_t[:], in_=idxs[:])
            dstl_t = cpool.tile([P, ncolt], f32, name="dstl_t")
            nc.sync.dma_start(out=dstl_t[:], in_=dstl[:])
            dd_t = cpool.tile([P, NBLK], f32, name="dd_t")
            nc.sync.dma_start(out=dd_t[:], in_=dinv_dst[:])

            rep_cm = tc.For_i(0, reps, 1) if reps > 1 else None
            if rep_cm is not None:
                rep_cm.__enter__()

            # ---- dense: htab = fp16(xT^T @ W) ----
            with tc.tile_pool(name="psD", bufs=4, space="PSUM") as psD:
              if parts in ("full", "dense"):
                for off in range(0, N, CHW):
                    w = min(CHW, N - off)
                    xc = xpool.tile([P, CHW], f16, name="xc", tag="xc")
                    nc.sync.dma_start(out=xc[:, :w], in_=xT[:, off:off + w])
                    for t0 in range(0, w, QUAD * 128):
                        nt = min(QUAD * 128, w - t0)
                        nq = (nt + 127) // 128
                        hq = hpool.tile([P, QUAD, P], f16, name="hq", tag="hq")
                        for q in range(nq):
                            rows = min(128, nt - q * 128)
                            ps = psD.tile([P, P], f32, name="ps", tag="ps")
                            nc.tensor.matmul(
                                ps[:rows, :],
                                lhsT=xc[:, t0 + q * 128:t0 + q * 128 + rows],
                                rhs=W_t[:], start=True, stop=True)
                            nc.scalar.copy(out=hq[:rows, q, :],
                                           in_=ps[:rows, :])
                        r0 = off + t0
                        rr = min(QUAD * 128, N - r0)
                        if rr % 128 == 0:
                            nc.sync.dma_start(
                                out=htab[r0:r0 + rr, :].rearrange(
                                    "(q p) f -> p q f", p=P),
                            in_=hq[:, :nq, :])
                        else:
                            for q in range(nq):
                                rows = min(128, rr - q * 128)
                                nc.sync.dma_start(
                                    out=htab[r0 + q * 128:
                                             r0 + q * 128 + rows, :],
                                    in_=hq[:rows, q, :])

            # ---- aggregation ----
            with tc.tile_pool(name="psA", bufs=1, space="PSUM") as psA:
              if parts in ("full", "agg", "gather", "gatherfix", "gmm", "gmeq", "gb64"):
                for st in range(NST):
                    blocks = list(range(st * SB, min(st * SB + SB, NBLK)))
                    pstiles = {}
                    for i, b in enumerate(blocks):
                        pstiles[b] = psA.tile([P, P], f32, name="psA",
                                              tag=f"psA{i}")
                    for gi, (base, rlen, c0, ncols) in enumerate(instrs):
                        if colblk[c0] // SB != st:
                            continue
                        gt = gpool.tile([P, GC, P], f16, name="gt", tag="gt")
                        if parts == "gb64":
                            if gi < 64:
                                nc.gpsimd.dma_gather(
                                    gt[:], htab[0:RS, :],
                                    idxs_t[:, 0:GC * 8],
                                    GC * 128, GC * 128, P, queue_num=gi % 4)
                            continue
                        if parts == "gatherfix":
                            nc.gpsimd.dma_gather(
                                gt[:], htab[0:RS, :], idxs_t[:, 0:GC * 8],
                                GC * 128, GC * 128, P, queue_num=gi % 4)
                            continue
                        nc.gpsimd.dma_gather(
                            gt[:, :ncols, :], htab[base:base + rlen, :],
                            idxs_t[:, c0 * 8:(c0 + ncols) * 8],
                            ncols * 128, ncols * 128, P, queue_num=gi % 4)
                        for k in range(ncols):
                            if parts in ("gather", "gatherfix"):
                                continue
                            j = c0 + k
                            b = int(colblk[j])
                            if parts != "gmm":
                                M = mpool.tile([P, P], f16, name="M",
                                               tag="M")
                                nc.vector.tensor_scalar(
                                    out=M[:], in0=iota_t[:],
                                    scalar1=dstl_t[:, j:j + 1], scalar2=None,
                                    op0=mybir.AluOpType.is_equal)
                            if parts == "gmeq":
                                continue
                            nc.tensor.matmul(
                                pstiles[b][:],
                                lhsT=(iota_t[:] if parts == "gmm" else M[:]),
                                rhs=gt[:, k, :],
                                start=(j == first_col[b]),
                                stop=(j == last_col[b]))
                    for b in (blocks if parts in ("full", "agg") else []):
                        rows = min(128, NPC - b * 128)
                        ob = opool.tile([P, P], f32, name="ob", tag="ob")
                        nc.vector.tensor_scalar(
                            out=ob[:], in0=pstiles[b][:],
                            scalar1=dd_t[:, b:b + 1], scalar2=None,
                            op0=mybir.AluOpType.mult)
                        nc.vector.tensor_tensor(
                            out=ob[:], in0=ob[:], in1=brep_t[:],
                            op=mybir.AluOpType.add)
                        nc.vector.tensor_scalar(
                            out=ob[:], in0=ob[:], scalar1=thr_t[:, :1],
                            scalar2=None, op0=mybir.AluOpType.max)
                        nc.sync.dma_start(out=out[b * 128:b * 128 + rows, :],
                                          in_=ob[:rows, :])
            if rep_cm is not None:
                rep_cm.__exit__(None, None, None)
    nc.compile()
    return nc


def _build_decode(dmeta, reps=1):
    """Decode: o[p, c] = sum_f z[ii[p,c], f] * z[jj[p,c], f]."""
    ncc = dmeta["ncc"]
    iinstr = dmeta["iinstr"]
    jinstr = dmeta["jinstr"]
    IW = ncc * 128 // 16
    F = OUT

    nc = bacc.Bacc("TRN2", target_bir_lowering=False, debug=False,
                   num_devices=NCORES, num_swdge_queues=4)
    z = nc.dram_tensor("z", [N, F], f32, kind="ExternalInput").ap()
    ii = nc.dram_tensor("ii", [P, IW], i16, kind="ExternalInput").ap()
    jj = nc.dram_tensor("jj", [P, IW], i16, kind="ExternalInput").ap()
    o = nc.dram_tensor("o", [P, ncc], f32, kind="ExternalOutput").ap()

    with tile.TileContext(nc) as tc:
        with (tc.tile_pool(name="c", bufs=1) as cpool,
              tc.tile_pool(name="gi", bufs=12) as gipool,
              tc.tile_pool(name="gj", bufs=12) as gjpool,
              tc.tile_pool(name="pr", bufs=12) as prpool):
            ii_t = cpool.tile([P, IW], i16, name="ii_t")
            nc.sync.dma_start(out=ii_t[:], in_=ii[:])
            jj_t = cpool.tile([P, IW], i16, name="jj_t")
            nc.sync.dma_start(out=jj_t[:], in_=jj[:])
            oc = cpool.tile([P, ncc], f32, name="oc")

            rep_cm = tc.For_i(0, reps, 1) if reps > 1 else None
            if rep_cm is not None:
                rep_cm.__enter__()
            # merge the two instruction streams in column order
            events = ([("i", t) for t in iinstr] + [("j", t) for t in jinstr])
            events.sort(key=lambda e: (e[1][2], 0 if e[0] == "i" else 1))
            qn = 0
            itiles = {}  # col -> (tile, col0)
            jtiles = {}
            done = set()
            for (kind, (base, rlen, c0, ncols)) in events:
                if kind == "i":
                    gt = gipool.tile([P, GC2, F], f32, name="git", tag="git")
                    nc.gpsimd.dma_gather(
                        gt[:, :ncols, :], z[base:base + rlen, :],
                        ii_t[:, c0 * 8:(c0 + ncols) * 8],
                        ncols * 128, ncols * 128, F, queue_num=qn % 4)
                    qn += 1
                    for k in range(ncols):
                        itiles[c0 + k] = (gt, k)
                else:
                    gt = gjpool.tile([P, GC2, F], f32, name="gjt", tag="gjt")
                    nc.gpsimd.dma_gather(
                        gt[:, :ncols, :], z[base:base + rlen, :],
                        jj_t[:, c0 * 8:(c0 + ncols) * 8],
                        ncols * 128, ncols * 128, F, queue_num=qn % 4)
                    qn += 1
                    for k in range(ncols):
                        jtiles[c0 + k] = (gt, k)
                ready = [c for c in itiles if c in jtiles and c not in done]
                for c in sorted(ready):
                    (git, ki) = itiles[c]
                    (gjt, kj) = jtiles[c]
                    pr = prpool.tile([P, F], f32, name="pr", tag="pr")
                    nc.vector.tensor_tensor(out=pr[:], in0=git[:, ki, :],
                                            in1=gjt[:, kj, :],
                                            op=mybir.AluOpType.mult)
                    nc.vector.tensor_reduce(
                        out=oc[:, c:c + 1], in_=pr[:],
                        axis=mybir.AxisListType.X, op=mybir.AluOpType.add)
                    done.add(c)
            if rep_cm is not None:
                rep_cm.__exit__(None, None, None)
            nc.sync.dma_start(out=o[:], in_=oc[:])
    nc.compile()
    return nc


def _get_programs(meta, dmeta):
    key = ("progs", meta["struct"], dmeta["struct"])
    if key not in _prog_cache:
        _prog_cache[key] = (_build_layer(meta), _build_decode(dmeta))
    return _prog_cache[key]


def _layer_maps(meta, xTv, Wv, brv, thv):
    iota = np.broadcast_to(np.arange(P, dtype=np.float16)[None, :],
                           (P, P)).copy()
    return [
        {"xT": xTv, "W": Wv, "brep": brv, "thr": thv, "iota": iota,
         "idxs": meta["idxts"][c], "dstl": meta["dstlts"][c],
         "dinv_dst": meta["dinv_dst"][c]}
        for c in range(NCORES)
    ]


def kernel(x, W1, b1, W2, b2, edge_index, edge_label_idx):
    x = np.asarray(x, dtype=np.float32)
    W1 = np.asarray(W1, dtype=np.float32)
    b1 = np.asarray(b1, dtype=np.float32)
    W2 = np.asarray(W2, dtype=np.float32)
    b2 = np.asarray(b2, dtype=np.float32)
    eidx = np.asarray(edge_index)

    meta = _prep(eidx)
    dmeta = _prep_decode(edge_label_idx)
    nc_layer, nc_dec = _get_programs(meta, dmeta)
    dinv = meta["dinv"]

    W1h = W1.astype(np.float16)
    W2h = np.zeros((P, P), np.float16)
    W2h[:, :OUT] = W2
    b1rep = np.broadcast_to(b1[None, :], (P, P)).astype(np.float32).copy()
    b2rep = np.zeros((P, P), np.float32)
    b2rep[:, :OUT] = b2[None, :]
    thr_relu = np.zeros((P, 1), np.float32)
    thr_id = np.full((P, 1), -1e30, np.float32)

    core_ids = list(range(NCORES))
    # layer 1
    x1T = np.ascontiguousarray((x * dinv[:, None]).astype(np.float16).T)
    res1 = run_bass_kernel_spmd(
        nc_layer, _layer_maps(meta, x1T, W1h, b1rep, thr_relu), core_ids)
    h1 = np.concatenate([res1.results[c]["out"] for c in range(NCORES)],
                        axis=0)
    # layer 2
    x2T = np.ascontiguousarray((h1 * dinv[:, None]).astype(np.float16).T)
    res2 = run_bass_kernel_spmd(
        nc_layer, _layer_maps(meta, x2T, W2h, b2rep, thr_id), core_ids)
    zfull = np.concatenate([res2.results[c]["out"] for c in range(NCORES)],
                           axis=0)
    z64 = np.ascontiguousarray(zfull[:, :OUT])
    # decode
    dec_maps = [{"z": z64, "ii": dmeta["iits"][c], "jj": dmeta["jjts"][c]}
                for c in range(NCORES)]
    res3 = run_bass_kernel_spmd(nc_dec, dec_maps, core_ids)
    out = np.empty(EL, np.float32)
    for c in range(NCORES):
        flat = res3.results[c]["o"].T.reshape(-1)
        out[c * PPC + dmeta["origs"][c]] = flat[dmeta["restore"][c]]
    return out.astype(np.float32)
